# revision 6
# baseline (speedup 1.0000x reference)
"""nn_CrossAttention kernel — data-parallel over batch B=8 across 8 NeuronCores.

Takes FULL unsharded inputs, returns FULL output [8, 64, 64, 512] float32.

Wall-clock is dominated by the axon tunnel (~70 MB/s each way, full duplex,
~0.1 s per-call issue latency), so the strategy is transfer-minimal and
pipelined:
  - quantize x1/x2 to uint8 with per-token scales on host (rel-err ~8e-3,
    budget 2e-2); pack x1q|x2q|scales into ONE uint8 array per device
  - each core computes only the attention branch; the residual `x1 +` is done
    on host in fp32 from the original x1
  - download the attention output as uint8 + per-token fp16 scales (2.1 MB
    per core instead of 8.4 MB fp32)
  - per-device worker threads pipeline quantize -> upload -> compute ->
    download -> dequant+add, so downloads of early cores overlap uploads of
    later cores (duplex tunnel)
  - weights are device-cached across calls
"""

import threading
import numpy as np

B, H, W = 8, 64, 64
D = 256
HEADS = 8
DK = D // HEADS
N = H * W
EPS = 1e-5

PACK_W = 2 * D + D + 4          # x1q 512 | x2q 256 | s1 fp16 | s2 fp16
OUT_W = 2 * D + 2               # attq 512 | sa fp16

_STATE = {}


def _get_state():
    if _STATE:
        return _STATE
    import jax
    import jax.numpy as jnp

    devs = jax.devices()[:8]
    assert len(devs) == 8

    def attn_fn(ucat, lw, lb, g1, b1, rw, rb):
        # ucat [N, PACK_W] uint8; weights fp32.
        s1 = jax.lax.bitcast_convert_type(
            ucat[:, 3 * D:3 * D + 2], jnp.float16).astype(jnp.float32)
        s2 = jax.lax.bitcast_convert_type(
            ucat[:, 3 * D + 2:3 * D + 4], jnp.float16).astype(jnp.float32)
        x1f = (ucat[:, :2 * D].astype(jnp.float32) - 127.0) * s1[:, None]
        x2f = (ucat[:, 2 * D:3 * D].astype(jnp.float32) - 127.0) * s2[:, None]

        def _ln(x, g, bb):
            m = jnp.mean(x, axis=-1, keepdims=True)
            v = jnp.var(x, axis=-1, keepdims=True)
            return (x - m) * jax.lax.rsqrt(v + EPS) * g + bb

        n1 = _ln(x1f @ lw + lb, g1, b1)              # [N, D]
        n2 = _ln(x2f, g1, b1)                        # [N, D]
        v = n1.T.reshape(HEADS, DK, N)
        kq = n2.T.reshape(HEADS, DK, N)
        k = jax.nn.softmax(kq, axis=-1)
        q = jax.nn.softmax(kq, axis=1)
        ctx = jnp.einsum('hdm,hem->hde', q, k)
        att = jnp.einsum('hde,hen->hdn', ctx, v)
        agg = att.reshape(D, N)                      # [D, N]
        rep = rw @ agg + rb[:, None]                 # [2D, N]
        out = _ln(rep.T, 1.0, 0.0)                   # [N, 2D]

        sa = jnp.max(jnp.abs(out), axis=1) / 127.0   # [N]
        ua = jnp.clip(jnp.rint(out / sa[:, None]) + 127.0, 0, 254
                      ).astype(jnp.uint8)
        sab = jax.lax.bitcast_convert_type(sa.astype(jnp.float16), jnp.uint8)
        return jnp.concatenate([ua, sab], axis=1)    # [N, OUT_W] uint8

    jf = jax.jit(attn_fn)
    _STATE['jax'] = jax
    _STATE['devs'] = devs
    _STATE['fn'] = jf
    _STATE['wcache'] = {}
    return _STATE


def _device_weights(st, inputs):
    """device_put the (tiny) weights once per distinct weight set."""
    jax = st['jax']
    names = ('linear_w', 'linear_b', 'ln1_g', 'ln1_b', 'reproj_w', 'reproj_b')
    key = tuple(int(np.asarray(inputs[n]).view(np.uint32).sum()) for n in names)
    cached = st['wcache'].get(key)
    if cached is not None:
        return cached
    per_dev = []
    for d in st['devs']:
        per_dev.append(tuple(
            jax.device_put(np.asarray(inputs[n], np.float32), d)
            for n in names))
    st['wcache'] = {key: per_dev}
    return per_dev


def _quant_pack(x1i, x2i, buf):
    """uint8-quantize one shard into buf [N, PACK_W] with per-token scales."""
    mx1 = np.abs(x1i).max(axis=1)
    mx2 = np.abs(x2i).max(axis=1)
    s1 = (mx1 / 127.0).astype(np.float16)
    s2 = (mx2 / 127.0).astype(np.float16)
    r1 = 127.0 / np.maximum(mx1, 1e-20)
    r2 = 127.0 / np.maximum(mx2, 1e-20)
    t1 = x1i * r1[:, None]
    t1 += 127.5
    buf[:, :2 * D] = t1.astype(np.uint8)
    t2 = x2i * r2[:, None]
    t2 += 127.5
    buf[:, 2 * D:3 * D] = t2.astype(np.uint8)
    buf[:, 3 * D:3 * D + 2] = s1[:, None].view(np.uint8)
    buf[:, 3 * D + 2:3 * D + 4] = s2[:, None].view(np.uint8)


def _kernel_trn(inputs):
    st = _get_state()
    jax = st['jax']
    devs = st['devs']
    fn = st['fn']

    x1 = np.asarray(inputs['x1'], np.float32)
    x2 = np.asarray(inputs['x2'], np.float32)
    wts = _device_weights(st, inputs)

    x1f = x1.reshape(B, N, 2 * D)
    x2f = x2.reshape(B, N, D)

    out = np.empty((B, N, 2 * D), np.float32)
    errs = []
    # Ladder on the host-side quantization only: it serializes on the GIL
    # anyway, and doing it in device order makes put issue order (and thus
    # tunnel FIFO completion order) deterministic 0..7.
    conv_done = [threading.Event() for _ in range(B)]

    def dev_worker(i):
        try:
            if i > 0:
                conv_done[i - 1].wait()
            buf = np.empty((N, PACK_W), np.uint8)
            _quant_pack(x1f[i], x2f[i], buf)
            conv_done[i].set()
            xh = jax.device_put(buf, devs[i])
            ret = np.asarray(fn(xh, *wts[i]))
            sa = ret[:, 2 * D:2 * D + 2].copy().view(np.float16)
            att = (ret[:, :2 * D].astype(np.float32) - 127.0)
            att *= sa.astype(np.float32)
            np.add(x1f[i], att, out=out[i])
        except Exception as e:  # noqa: BLE001
            errs.append(e)
            conv_done[i].set()

    threads = [threading.Thread(target=dev_worker, args=(i,))
               for i in range(B)]
    for t in threads:
        t.start()
    for t in threads:
        t.join()
    if errs:
        raise errs[0]
    return out.reshape(B, H, W, 2 * D)


def _kernel_numpy(inputs):
    """CPU fallback, exact reference math in float32."""
    x1 = np.asarray(inputs['x1'], np.float32)
    x2 = np.asarray(inputs['x2'], np.float32)
    lw = np.asarray(inputs['linear_w'], np.float32)
    lb = np.asarray(inputs['linear_b'], np.float32)
    g1 = np.asarray(inputs['ln1_g'], np.float32)
    b1 = np.asarray(inputs['ln1_b'], np.float32)
    rw = np.asarray(inputs['reproj_w'], np.float32)
    rb = np.asarray(inputs['reproj_b'], np.float32)

    def _ln(x, g, bb):
        m = x.mean(-1, keepdims=True)
        v = x.var(-1, keepdims=True)
        return (x - m) / np.sqrt(v + EPS) * g + bb

    def _softmax(x, axis):
        x = x - x.max(axis=axis, keepdims=True)
        e = np.exp(x)
        return e / e.sum(axis=axis, keepdims=True)

    n1 = _ln(x1 @ lw + lb, g1, b1)
    n2 = _ln(x2, g1, b1)
    v = n1.reshape(B, N, D).transpose(0, 2, 1).reshape(B, HEADS, DK, N)
    kq = n2.reshape(B, N, D).transpose(0, 2, 1).reshape(B, HEADS, DK, N)
    k = _softmax(kq, -1)
    q = _softmax(kq, 2)
    ctx = np.einsum('bhdm,bhem->bhde', q, k)
    att = np.einsum('bhde,bhen->bhdn', ctx, v)
    agg = att.reshape(B, D, H, W)
    rep = np.einsum('od,bdhw->bohw', rw, agg) + rb[None, :, None, None]
    rep = rep.transpose(0, 2, 3, 1)
    return (x1 + _ln(rep, np.ones(2 * D, np.float32),
                     np.zeros(2 * D, np.float32))).astype(np.float32)


def kernel(**inputs):
    try:
        return _kernel_trn(inputs)
    except Exception:
        return _kernel_numpy(inputs)


# revision 11
# speedup vs baseline: 29.2047x; 29.2047x over previous
"""nn_CrossAttention kernel — data-parallel over batch B=8 across 8 NeuronCores.

Takes FULL unsharded inputs, returns FULL output [8, 64, 64, 512] float32.

Wall-clock is dominated by the axon tunnel (~70 MB/s each way, full duplex,
~0.1 s per-call issue latency), so the strategy is transfer-minimal and
pipelined:
  - quantize x1/x2 to uint8 with per-token scales on host (rel-err ~8e-3,
    budget 2e-2); pack x1q|x2q|scales into ONE uint8 array per device
  - each core computes only the attention branch; the residual `x1 +` is done
    on host in fp32 from the original x1
  - download the attention output as uint8 + per-token fp16 scales (2.1 MB
    per core instead of 8.4 MB fp32)
  - per-device worker threads pipeline quantize -> upload -> compute ->
    download -> dequant+add, so downloads of early cores overlap uploads of
    later cores (duplex tunnel)
  - weights are device-cached across calls
"""

import threading
import numpy as np

B, H, W = 8, 64, 64
D = 256
HEADS = 8
DK = D // HEADS
N = H * W
EPS = 1e-5

PACK_W = 2 * D + D + 4          # x1q 512 | x2q 256 | s1 fp16 | s2 fp16
OUT_W = 2 * D + 2               # attq 512 | sa fp16

_STATE = {}


def _get_state():
    if _STATE:
        return _STATE
    import jax
    import jax.numpy as jnp

    devs = jax.devices()[:8]
    assert len(devs) == 8

    def attn_fn(ucat, lw, lb, g1, b1, rw, rb):
        # ucat [N, PACK_W] uint8; weights fp32. Scales ride along as
        # fixed-point uint16 (value * 2^20) split into two uint8 columns —
        # neuronx-cc crashes on bitcast_convert_type in this graph, so the
        # codec is pure arithmetic.
        def dec(c0):
            return (ucat[:, c0].astype(jnp.float32)
                    + 256.0 * ucat[:, c0 + 1].astype(jnp.float32)) * 2.0**-20
        s1 = dec(3 * D)
        s2 = dec(3 * D + 2)
        x1f = (ucat[:, :2 * D].astype(jnp.float32) - 127.0) * s1[:, None]
        x2f = (ucat[:, 2 * D:3 * D].astype(jnp.float32) - 127.0) * s2[:, None]

        def _ln(x, g, bb):
            m = jnp.mean(x, axis=-1, keepdims=True)
            v = jnp.var(x, axis=-1, keepdims=True)
            return (x - m) * jax.lax.rsqrt(v + EPS) * g + bb

        n1 = _ln(x1f @ lw + lb, g1, b1)              # [N, D]
        n2 = _ln(x2f, g1, b1)                        # [N, D]
        v = n1.T.reshape(HEADS, DK, N)
        kq = n2.T.reshape(HEADS, DK, N)
        k = jax.nn.softmax(kq, axis=-1)
        q = jax.nn.softmax(kq, axis=1)
        ctx = jnp.einsum('hdm,hem->hde', q, k)
        att = jnp.einsum('hde,hen->hdn', ctx, v)
        agg = att.reshape(D, N)                      # [D, N]
        rep = rw @ agg + rb[:, None]                 # [2D, N]
        out = _ln(rep.T, 1.0, 0.0)                   # [N, 2D]

        sa = jnp.max(jnp.abs(out), axis=1) / 127.0   # [N]
        ua = jnp.clip(jnp.rint(out / sa[:, None]) + 127.0, 0, 254
                      ).astype(jnp.uint8)
        sau = jnp.rint(sa * 2.0**20)
        q0 = jnp.mod(sau, 256.0).astype(jnp.uint8)[:, None]
        q1 = jnp.floor(sau / 256.0).astype(jnp.uint8)[:, None]
        return jnp.concatenate([ua, q0, q1], axis=1)  # [N, OUT_W] uint8

    jf = jax.jit(attn_fn)
    _STATE['jax'] = jax
    _STATE['devs'] = devs
    _STATE['fn'] = jf
    _STATE['wcache'] = {}
    return _STATE


def _device_weights(st, inputs):
    """device_put the (tiny) weights once per distinct weight set."""
    jax = st['jax']
    names = ('linear_w', 'linear_b', 'ln1_g', 'ln1_b', 'reproj_w', 'reproj_b')
    key = tuple(int(np.asarray(inputs[n]).view(np.uint32).sum()) for n in names)
    cached = st['wcache'].get(key)
    if cached is not None:
        return cached
    per_dev = []
    for d in st['devs']:
        per_dev.append(tuple(
            jax.device_put(np.asarray(inputs[n], np.float32), d)
            for n in names))
    st['wcache'] = {key: per_dev}
    return per_dev


def _quant_pack(x1i, x2i, buf):
    """uint8-quantize one shard into buf [N, PACK_W] with per-token scales.

    Scales are encoded as uint16 fixed-point (scale * 2^20); quantization
    uses exactly the decoded scale so host and device agree.
    """
    mx1 = np.abs(x1i).max(axis=1)
    mx2 = np.abs(x2i).max(axis=1)
    s1u = np.maximum((mx1 * (2.0**20 / 127.0)), 1.0).astype(np.uint16)
    s2u = np.maximum((mx2 * (2.0**20 / 127.0)), 1.0).astype(np.uint16)
    r1 = 2.0**20 / s1u.astype(np.float32)
    r2 = 2.0**20 / s2u.astype(np.float32)
    t1 = x1i * r1[:, None]
    t1 += 127.5
    buf[:, :2 * D] = t1.astype(np.uint8)
    t2 = x2i * r2[:, None]
    t2 += 127.5
    buf[:, 2 * D:3 * D] = t2.astype(np.uint8)
    buf[:, 3 * D:3 * D + 2] = s1u[:, None].view(np.uint8)
    buf[:, 3 * D + 2:3 * D + 4] = s2u[:, None].view(np.uint8)


def _kernel_trn(inputs):
    st = _get_state()
    jax = st['jax']
    devs = st['devs']
    fn = st['fn']

    x1 = np.asarray(inputs['x1'], np.float32)
    x2 = np.asarray(inputs['x2'], np.float32)
    wts = _device_weights(st, inputs)

    x1f = x1.reshape(B, N, 2 * D)
    x2f = x2.reshape(B, N, D)

    out = np.empty((B, N, 2 * D), np.float32)
    errs = []
    # Ladder on the host-side quantization only: it serializes on the GIL
    # anyway, and doing it in device order makes put issue order (and thus
    # tunnel FIFO completion order) deterministic 0..7.
    conv_done = [threading.Event() for _ in range(B)]

    def dev_worker(i):
        try:
            if i > 0:
                conv_done[i - 1].wait()
            buf = np.empty((N, PACK_W), np.uint8)
            _quant_pack(x1f[i], x2f[i], buf)
            conv_done[i].set()
            xh = jax.device_put(buf, devs[i])
            ret = np.asarray(fn(xh, *wts[i]))
            sau = ret[:, 2 * D:2 * D + 2].copy().view(np.uint16)
            sa = sau.astype(np.float32) * 2.0**-20
            att = (ret[:, :2 * D].astype(np.float32) - 127.0)
            att *= sa
            np.add(x1f[i], att, out=out[i])
        except Exception as e:  # noqa: BLE001
            errs.append(e)
            conv_done[i].set()

    threads = [threading.Thread(target=dev_worker, args=(i,))
               for i in range(B)]
    for t in threads:
        t.start()
    for t in threads:
        t.join()
    if errs:
        raise errs[0]
    return out.reshape(B, H, W, 2 * D)


def _kernel_numpy(inputs):
    """CPU fallback, exact reference math in float32."""
    x1 = np.asarray(inputs['x1'], np.float32)
    x2 = np.asarray(inputs['x2'], np.float32)
    lw = np.asarray(inputs['linear_w'], np.float32)
    lb = np.asarray(inputs['linear_b'], np.float32)
    g1 = np.asarray(inputs['ln1_g'], np.float32)
    b1 = np.asarray(inputs['ln1_b'], np.float32)
    rw = np.asarray(inputs['reproj_w'], np.float32)
    rb = np.asarray(inputs['reproj_b'], np.float32)

    def _ln(x, g, bb):
        m = x.mean(-1, keepdims=True)
        v = x.var(-1, keepdims=True)
        return (x - m) / np.sqrt(v + EPS) * g + bb

    def _softmax(x, axis):
        x = x - x.max(axis=axis, keepdims=True)
        e = np.exp(x)
        return e / e.sum(axis=axis, keepdims=True)

    n1 = _ln(x1 @ lw + lb, g1, b1)
    n2 = _ln(x2, g1, b1)
    v = n1.reshape(B, N, D).transpose(0, 2, 1).reshape(B, HEADS, DK, N)
    kq = n2.reshape(B, N, D).transpose(0, 2, 1).reshape(B, HEADS, DK, N)
    k = _softmax(kq, -1)
    q = _softmax(kq, 2)
    ctx = np.einsum('bhdm,bhem->bhde', q, k)
    att = np.einsum('bhde,bhen->bhdn', ctx, v)
    agg = att.reshape(B, D, H, W)
    rep = np.einsum('od,bdhw->bohw', rw, agg) + rb[None, :, None, None]
    rep = rep.transpose(0, 2, 3, 1)
    return (x1 + _ln(rep, np.ones(2 * D, np.float32),
                     np.zeros(2 * D, np.float32))).astype(np.float32)


def kernel(**inputs):
    try:
        return _kernel_trn(inputs)
    except Exception:
        return _kernel_numpy(inputs)
